# revision 6
# baseline (speedup 1.0000x reference)
"""Triplet-margin loss (EuclideanTriple) on 8 Trainium2 NeuronCores.

loss = sum_i relu( ||x_i - y_i + eps||_2 + margin - ||x_i - z_i + eps||_2 )

Data-parallel: N=131072 rows sharded 8 ways (16384 rows/core, no
collectives). HBM traffic is the roofline, so inputs stream as bf16
(host casts f32 -> bf16): 24 MiB/core instead of 48. rel-err budget is
2e-2; bf16 end-to-end error is ~1e-4.

Layout: the host pre-arranges each core's shard into contiguous SBUF
tile images [n_blocks*128, F] where block (chunk c, d-half h) holds
rows c*F..c*F+F transposed so the feature dim d sits on partitions.
Each DMA is then one contiguous span (full HBM streaming efficiency),
and the per-row d-reduction runs on the otherwise-idle TensorEngine as
a ones-matmul into PSUM: psum[m, r] = sum_p sq[p, r].

Per compute slice of G=1024 rows:
  DVE : u = x - y, v = x - z (tensor_sub in place, bf16 2x mode)
        + squares of k_dve_sq of the 4 half-tiles (tensor_mul, 2x)
  ACT : squares of the remaining half-tiles (Square, 1x)
  PE  : ones[128,32]^T @ sq -> psum_p/psum_n [32, G] f32 (2 matmuls
        per 512-col PSUM bank, accumulating the two d-halves)
  ACT : dp = Sqrt(psum_p); dn = Sqrt(psum_n), with accum_out column
        dnacc[:, idx] = sum_r dn  (rides free on the sqrt)
  DVE : scalar_tensor_tensor: scratch = max(dp, dn - margin) with
        accum_out column hracc[:, idx]  (one fused op, using the
        identity max(dp-dn, -m) = max(dp, dn-m) - dn)
Host: loss = sum_cores [ sum(hracc row0) - sum(dnacc row0) ] + m*N
(exact identity sum relu(h+m) = sum max(h,-m) + m*N).

All reduction tiles are partition-duplicated [32, *]; host reads row 0.
"""

from contextlib import ExitStack

import numpy as np
import ml_dtypes

import concourse.bacc as bacc
import concourse.bass as bass
import concourse.mybir as mybir
import concourse.tile as tile
from concourse import bass_utils

N_TOTAL = 131072
D = 256
N_CORES = 8
SHARD = N_TOTAL // N_CORES  # 16384 rows per core
P = 128                     # SBUF partitions (one d-half)
F = 4096                    # rows per DMA chunk
G = 1024                    # rows per compute slice
MARGIN = 0.5
ONES_M = 32                 # duplicated output partitions
BF16 = mybir.dt.bfloat16
F32 = mybir.dt.float32
BANK = 512                  # f32 elems per PSUM bank


def build_nc(
    repeat: int = 1,
    mode: str = "full",
    loop: bool = False,
    io_bufs: int = 2,
    k_dve_sq: int = 2,
    gp_sub: int = 1,
    chunk_f: int = F,
    slice_g: int = G,
) -> bass.Bass:
    """mode: 'full' | 'dma' | 'sub' | 'sq' | 'red'.
    gp_sub: how many of the 4 per-slice tensor_subs run on GPSIMD.
    loop=True wraps repeats in a For_i hardware loop for timing runs."""
    n_chunks = SHARD // chunk_f
    n_g = chunk_f // slice_g
    n_cols = n_chunks * n_g  # accumulator columns per pass
    nc = bacc.Bacc("TRN2", target_bir_lowering=False, debug=False)
    x = nc.dram_tensor("x", [n_chunks * 2 * P, chunk_f], BF16, kind="ExternalInput").ap()
    y = nc.dram_tensor("y", [n_chunks * 2 * P, chunk_f], BF16, kind="ExternalInput").ap()
    z = nc.dram_tensor("z", [n_chunks * 2 * P, chunk_f], BF16, kind="ExternalInput").ap()
    out = nc.dram_tensor("out", [ONES_M, 2 * n_cols], F32, kind="ExternalOutput").ap()

    act = mybir.ActivationFunctionType
    alu = mybir.AluOpType

    with tile.TileContext(nc) as tc:
        with ExitStack() as ctx:
            io = ctx.enter_context(tc.tile_pool(name="io", bufs=io_bufs))
            dd = ctx.enter_context(tc.tile_pool(name="dd", bufs=3))
            ps = ctx.enter_context(tc.tile_pool(name="ps", bufs=2, space="PSUM"))
            accp = ctx.enter_context(tc.tile_pool(name="accp", bufs=2))
            single = ctx.enter_context(tc.tile_pool(name="single", bufs=1))

            ones = single.tile([P, ONES_M], BF16, tag="ones")
            nc.vector.memset(ones[:], 1.0)

            io_tags = ("xlo", "xhi", "ylo", "yhi", "zlo", "zhi")

            def rep_body():
                hracc = accp.tile([ONES_M, n_cols], F32, tag="hracc", name="hracc")
                dnacc = accp.tile([ONES_M, n_cols], F32, tag="dnacc", name="dnacc")
                for c in range(n_chunks):
                    tiles = {
                        tag: io.tile([P, chunk_f], BF16, tag=tag, name=tag)
                        for tag in io_tags
                    }
                    for i, (tag, src) in enumerate(
                        (("xlo", x), ("xhi", x), ("ylo", y),
                         ("yhi", y), ("zlo", z), ("zhi", z))
                    ):
                        blk = 2 * c + (i % 2)
                        nc.sync.dma_start(
                            tiles[tag][:], src[blk * P : (blk + 1) * P, :]
                        )
                    if mode == "dma":
                        continue
                    xlo, xhi = tiles["xlo"], tiles["xhi"]
                    ulo, uhi = tiles["ylo"], tiles["yhi"]
                    vlo, vhi = tiles["zlo"], tiles["zhi"]
                    for g in range(n_g):
                        sl = slice(g * slice_g, (g + 1) * slice_g)
                        idx = c * n_g + g
                        # u = x - y, v = x - z (in place into y/z tiles)
                        pairs = (
                            (ulo, xlo), (uhi, xhi), (vlo, xlo), (vhi, xhi)
                        )
                        for j, (t, xt) in enumerate(pairs):
                            eng = nc.gpsimd if j < gp_sub else nc.vector
                            eng.tensor_sub(t[:, sl], xt[:, sl], t[:, sl])
                        if mode == "sub":
                            continue
                        # squares in place; first k on DVE (2x), rest ACT
                        for j, t in enumerate((ulo, vlo, uhi, vhi)):
                            if j < k_dve_sq:
                                nc.vector.tensor_mul(t[:, sl], t[:, sl], t[:, sl])
                            else:
                                nc.scalar.activation(t[:, sl], t[:, sl], act.Square)
                        if mode == "sq":
                            continue
                        # d-reduction on the TensorEngine
                        pp = ps.tile([ONES_M, slice_g], F32, tag="pp", name="pp")
                        pn = ps.tile([ONES_M, slice_g], F32, tag="pn", name="pn")
                        for s in range(slice_g // BANK):
                            bs = slice(s * BANK, (s + 1) * BANK)
                            gs = slice(g * slice_g + s * BANK, g * slice_g + (s + 1) * BANK)
                            nc.tensor.matmul(
                                pp[:, bs], ones[:], ulo[:, gs], start=True, stop=False
                            )
                            nc.tensor.matmul(
                                pp[:, bs], ones[:], uhi[:, gs], start=False, stop=True
                            )
                            nc.tensor.matmul(
                                pn[:, bs], ones[:], vlo[:, gs], start=True, stop=False
                            )
                            nc.tensor.matmul(
                                pn[:, bs], ones[:], vhi[:, gs], start=False, stop=True
                            )
                        dp = dd.tile([ONES_M, slice_g], BF16, tag="dp", name="dp")
                        dn = dd.tile([ONES_M, slice_g], BF16, tag="dn", name="dn")
                        if mode == "red":
                            nc.scalar.activation(dp[:], pp[:], act.Copy)
                            nc.scalar.activation(dn[:], pn[:], act.Copy)
                            continue
                        nc.scalar.activation(dp[:], pp[:], act.Sqrt)
                        nc.scalar.activation(
                            dn[:], pn[:], act.Sqrt,
                            accum_out=dnacc[:, idx : idx + 1],
                        )
                        # scratch = max(dp, dn - margin); hracc col = sum
                        scr = dd.tile([ONES_M, slice_g], BF16, tag="scr", name="scr")
                        nc.vector.scalar_tensor_tensor(
                            scr[:], dn[:], -MARGIN, dp[:],
                            op0=alu.add, op1=alu.max,
                            accum_out=hracc[:, idx : idx + 1],
                        )
                if mode in ("dma", "sub", "sq", "red"):
                    return
                nc.sync.dma_start(out[:, 0:n_cols], hracc[:])
                nc.sync.dma_start(out[:, n_cols : 2 * n_cols], dnacc[:])

            if loop and repeat > 1:
                with tc.For_i(0, repeat, 1):
                    rep_body()
            else:
                for _ in range(repeat):
                    rep_body()
    nc.compile()
    return nc


def make_in_maps(x: np.ndarray, y: np.ndarray, z: np.ndarray, chunk_f: int = F):
    """Cast f32 -> bf16 and lay each core's shard out as contiguous SBUF
    tile images: [n_chunks*2*P, chunk_f], block (c, half) = transposed
    [128, chunk_f] slab of rows c*F..(c+1)*F, feature-half `half`."""
    bf = ml_dtypes.bfloat16
    n_chunks = SHARD // chunk_f
    maps = []
    for i in range(N_CORES):
        rows = slice(i * SHARD, (i + 1) * SHARD)
        m = {}
        for nm, arr in (("x", x), ("y", y), ("z", z)):
            b = arr[rows].astype(bf)                      # [SHARD, 256]
            b = b.reshape(n_chunks, chunk_f, 2, P)        # [c, f, half, p]
            b = np.ascontiguousarray(b.transpose(0, 2, 3, 1))  # [c, half, p, f]
            m[nm] = b.reshape(n_chunks * 2 * P, chunk_f)
        maps.append(m)
    return maps


_NC_CACHE = None


def kernel(x: np.ndarray, y: np.ndarray, z: np.ndarray) -> np.ndarray:
    global _NC_CACHE
    x = np.asarray(x, dtype=np.float32)
    y = np.asarray(y, dtype=np.float32)
    z = np.asarray(z, dtype=np.float32)
    if _NC_CACHE is None:
        _NC_CACHE = build_nc(1)
    res = bass_utils.run_bass_kernel_spmd(
        _NC_CACHE, make_in_maps(x, y, z), core_ids=list(range(N_CORES))
    )
    n_cols = (SHARD // F) * (F // G)
    total = np.float64(0.0)
    for r in res.results:
        row = r["out"][0].astype(np.float64)
        total += row[:n_cols].sum() - row[n_cols:].sum()
    # sum_i relu(h_i + m) == sum_i max(h_i, -m) + m*N  (exact identity)
    total += np.float64(MARGIN) * N_TOTAL
    return np.float32(total)


# revision 33
# speedup vs baseline: 1.3553x; 1.3553x over previous
"""Triplet-margin loss (EuclideanTriple) on 8 Trainium2 NeuronCores.

loss = sum_i relu( ||x_i - y_i + eps||_2 + margin - ||x_i - z_i + eps||_2 )

Data-parallel: N=131072 rows sharded 8 ways (16384 rows/core, no
collectives). HBM traffic is the roofline, so inputs stream as bf16
(host casts f32 -> bf16): 24 MiB/core instead of 48. rel-err budget is
2e-2; bf16 end-to-end error is ~1e-4.

Layout: the host pre-arranges each core's shard into contiguous SBUF
tile images [n_blocks*128, F] where block (chunk c, d-half h) holds
rows c*F..c*F+F transposed so the feature dim d sits on partitions.
Each DMA is then one contiguous span (full HBM streaming efficiency),
and the per-row d-reduction runs on the otherwise-idle TensorEngine as
a ones-matmul into PSUM: psum[m, r] = sum_p sq[p, r].

Per compute slice of G=1024 rows:
  DVE : u = x - y, v = x - z (tensor_sub in place, bf16 2x mode)
        + squares of k_dve_sq of the 4 half-tiles (tensor_mul, 2x)
  ACT : squares of the remaining half-tiles (Square, 1x)
  PE  : ones[128,32]^T @ sq -> psum_p/psum_n [32, G] f32 (2 matmuls
        per 512-col PSUM bank, accumulating the two d-halves)
  ACT : dp = Sqrt(psum_p); dn = Sqrt(psum_n), with accum_out column
        dnacc[:, idx] = sum_r dn  (rides free on the sqrt)
  DVE : scalar_tensor_tensor: scratch = max(dp, dn - margin) with
        accum_out column hracc[:, idx]  (one fused op, using the
        identity max(dp-dn, -m) = max(dp, dn-m) - dn)
Host: loss = sum_cores [ sum(hracc row0) - sum(dnacc row0) ] + m*N
(exact identity sum relu(h+m) = sum max(h,-m) + m*N).

All reduction tiles are partition-duplicated [32, *]; host reads row 0.
"""

from contextlib import ExitStack

import numpy as np
import ml_dtypes

import concourse.bacc as bacc
import concourse.bass as bass
import concourse.mybir as mybir
import concourse.tile as tile
from concourse import bass_utils

N_TOTAL = 131072
D = 256
N_CORES = 8
SHARD = N_TOTAL // N_CORES  # 16384 rows per core
P = 128                     # SBUF partitions (one d-half)
F = 4096                    # rows per DMA chunk
G = 1024                    # rows per compute slice
MARGIN = 0.5
ONES_M = 32                 # duplicated output partitions
BF16 = mybir.dt.bfloat16
F32 = mybir.dt.float32
BANK = 512                  # f32 elems per PSUM bank


def build_nc(
    repeat: int = 1,
    mode: str = "full",
    loop: bool = False,
    io_bufs: int = 2,
    k_dve_sq: int = 2,
    gp_sub: int = 0,
    chunk_f: int = F,
    slice_g: int = G,
    dd_bufs: int = 3,
    ps_bufs: int = 2,
    act_dma: int = 0,
    in_dtype: str = "bf16",
    stt_late: int = 1,
) -> bass.Bass:
    """mode: 'full' | 'dma' | 'sub' | 'sq' | 'red'.
    gp_sub: how many of the 4 per-slice tensor_subs run on GPSIMD.
    loop=True wraps repeats in a For_i hardware loop for timing runs."""
    n_chunks = SHARD // chunk_f
    n_g = chunk_f // slice_g
    n_cols = n_chunks * n_g  # accumulator columns per pass
    fp8 = mybir.dt.float8e3
    dts = {
        "bf16": (BF16, BF16, BF16),
        "fp8e3": (fp8, fp8, fp8),
        "mixed": (BF16, fp8, fp8),
    }[in_dtype]
    nc = bacc.Bacc("TRN2", target_bir_lowering=False, debug=False)
    x = nc.dram_tensor("x", [n_chunks * 2 * P, chunk_f], dts[0], kind="ExternalInput").ap()
    y = nc.dram_tensor("y", [n_chunks * 2 * P, chunk_f], dts[1], kind="ExternalInput").ap()
    z = nc.dram_tensor("z", [n_chunks * 2 * P, chunk_f], dts[2], kind="ExternalInput").ap()
    src_dt = {"x": dts[0], "y": dts[1], "z": dts[2]}
    out = nc.dram_tensor(
        "out", [ONES_M, n_chunks + n_cols], F32, kind="ExternalOutput"
    ).ap()

    act = mybir.ActivationFunctionType
    alu = mybir.AluOpType

    with tile.TileContext(nc) as tc:
        with ExitStack() as ctx:
            io = ctx.enter_context(tc.tile_pool(name="io", bufs=io_bufs))
            dd = ctx.enter_context(tc.tile_pool(name="dd", bufs=dd_bufs))
            # dp/dn live until the pass-end stt ops -> one buf per chunk
            ddp = ctx.enter_context(
                tc.tile_pool(name="ddp", bufs=n_chunks if stt_late else dd_bufs)
            )
            ps = ctx.enter_context(tc.tile_pool(name="ps", bufs=ps_bufs, space="PSUM"))
            accp = ctx.enter_context(tc.tile_pool(name="accp", bufs=2))
            single = ctx.enter_context(tc.tile_pool(name="single", bufs=1))

            ones = single.tile([P, ONES_M], BF16, tag="ones")
            nc.vector.memset(ones[:], 1.0)

            io_tags = ("xlo", "xhi", "ylo", "yhi", "zlo", "zhi")

            def rep_body():
                hracc = accp.tile([ONES_M, n_cols], F32, tag="hracc", name="hracc")
                dnacc = accp.tile([ONES_M, n_cols], F32, tag="dnacc", name="dnacc")
                tail = []
                for c in range(n_chunks):
                    tiles = {
                        tag: io.tile([P, chunk_f], BF16, tag=tag, name=tag)
                        for tag in io_tags
                    }
                    for i, (tag, src) in enumerate(
                        (("xlo", x), ("xhi", x), ("ylo", y),
                         ("yhi", y), ("zlo", z), ("zhi", z))
                    ):
                        blk = 2 * c + (i % 2)
                        if src_dt[tag[0]] != BF16:
                            deng = nc.gpsimd  # SWDGE casts in the DMA path
                        else:
                            deng = nc.scalar if (act_dma and i % 2) else nc.sync
                        deng.dma_start(
                            tiles[tag][:], src[blk * P : (blk + 1) * P, :]
                        )
                    if mode == "dma":
                        continue
                    xlo, xhi = tiles["xlo"], tiles["xhi"]
                    ulo, uhi = tiles["ylo"], tiles["yhi"]
                    vlo, vhi = tiles["zlo"], tiles["zhi"]
                    # u = x - y, v = x - z (in place, full-chunk ops)
                    pairs = ((ulo, xlo), (uhi, xhi), (vlo, xlo), (vhi, xhi))
                    for j, (t, xt) in enumerate(pairs):
                        eng = nc.gpsimd if j < gp_sub else nc.vector
                        eng.tensor_sub(t[:], xt[:], t[:])
                    if mode == "sub":
                        continue
                    # squares in place; first k on DVE (2x), rest ACT
                    for j, t in enumerate((ulo, vlo, uhi, vhi)):
                        if j < k_dve_sq:
                            nc.vector.tensor_mul(t[:], t[:], t[:])
                        else:
                            nc.scalar.activation(t[:], t[:], act.Square)
                    if mode == "sq":
                        continue
                    dp = ddp.tile([ONES_M, chunk_f], BF16, tag="dp", name="dp")
                    dn = ddp.tile([ONES_M, chunk_f], BF16, tag="dn", name="dn")
                    for g in range(n_g):
                        sl = slice(g * slice_g, (g + 1) * slice_g)
                        idx = c * n_g + g
                        # d-reduction on the TensorEngine
                        pp = ps.tile([ONES_M, slice_g], F32, tag="pp", name="pp")
                        pn = ps.tile([ONES_M, slice_g], F32, tag="pn", name="pn")
                        for s in range(slice_g // BANK):
                            bs = slice(s * BANK, (s + 1) * BANK)
                            gs = slice(g * slice_g + s * BANK, g * slice_g + (s + 1) * BANK)
                            nc.tensor.matmul(
                                pp[:, bs], ones[:], ulo[:, gs], start=True, stop=False
                            )
                            nc.tensor.matmul(
                                pp[:, bs], ones[:], uhi[:, gs], start=False, stop=True
                            )
                            nc.tensor.matmul(
                                pn[:, bs], ones[:], vlo[:, gs], start=True, stop=False
                            )
                            nc.tensor.matmul(
                                pn[:, bs], ones[:], vhi[:, gs], start=False, stop=True
                            )
                        if mode == "red":
                            nc.scalar.activation(dp[:, sl], pp[:], act.Copy)
                            nc.scalar.activation(dn[:, sl], pn[:], act.Copy)
                            continue
                        nc.scalar.activation(dp[:, sl], pp[:], act.Sqrt)
                        if mode == "sqrtna":
                            nc.scalar.activation(dn[:, sl], pn[:], act.Sqrt)
                            continue
                        nc.scalar.activation(
                            dn[:, sl], pn[:], act.Sqrt,
                            accum_out=dnacc[:, idx : idx + 1],
                        )
                    if mode in ("red", "sqrtna", "sqrt"):
                        continue
                    if stt_late:
                        tail.append((c, dp, dn))
                    else:
                        scr = dd.tile([ONES_M, chunk_f], BF16, tag="scr", name="scr")
                        nc.vector.scalar_tensor_tensor(
                            scr[:], dn[:], -MARGIN, dp[:],
                            op0=alu.add, op1=alu.max,
                            accum_out=hracc[:, c : c + 1],
                        )
                if mode in ("dma", "sub", "sq", "red", "sqrtna", "sqrt"):
                    return
                # scratch = max(dp, dn - margin); hracc col = chunk sum.
                # Deferred to pass end: issued mid-pipeline on DVE's in-order
                # queue they would stall the next chunk's subs behind this
                # chunk's sqrts; here they fill DVE's idle DMA-wait gap at
                # the next iteration's start instead.
                for c, dp, dn in tail:
                    scr = dd.tile([ONES_M, chunk_f], BF16, tag="scr", name="scr")
                    nc.vector.scalar_tensor_tensor(
                        scr[:], dn[:], -MARGIN, dp[:],
                        op0=alu.add, op1=alu.max,
                        accum_out=hracc[:, c : c + 1],
                    )
                # out-DMAs ride the ACT HWDGE ring: the sync ring is FIFO,
                # so putting these there would stall the next pass's loads
                # behind this pass's full compute.
                nc.scalar.dma_start(out[:, 0:n_chunks], hracc[:, 0:n_chunks])
                nc.scalar.dma_start(out[:, n_chunks : n_chunks + n_cols], dnacc[:])

            if loop and repeat > 1:
                with tc.For_i(0, repeat, 1):
                    rep_body()
            else:
                for _ in range(repeat):
                    rep_body()
    nc.compile()
    return nc


def make_in_maps(
    x: np.ndarray, y: np.ndarray, z: np.ndarray,
    chunk_f: int = F, in_dtype: str = "bf16",
):
    """Cast f32 -> bf16 (or fp8_e3m4) and lay each core's shard out as
    contiguous SBUF tile images: [n_chunks*2*P, chunk_f], block (c, half)
    = transposed [128, chunk_f] slab of rows c*F..(c+1)*F, half `half`."""
    host_dts = {
        "bf16": (ml_dtypes.bfloat16,) * 3,
        "fp8e3": (ml_dtypes.float8_e3m4,) * 3,
        "mixed": (ml_dtypes.bfloat16, ml_dtypes.float8_e3m4, ml_dtypes.float8_e3m4),
    }[in_dtype]
    n_chunks = SHARD // chunk_f
    maps = []
    for i in range(N_CORES):
        rows = slice(i * SHARD, (i + 1) * SHARD)
        m = {}
        for (nm, arr), bf in zip((("x", x), ("y", y), ("z", z)), host_dts):
            b = arr[rows].astype(bf)                      # [SHARD, 256]
            b = b.reshape(n_chunks, chunk_f, 2, P)        # [c, f, half, p]
            b = np.ascontiguousarray(b.transpose(0, 2, 3, 1))  # [c, half, p, f]
            m[nm] = b.reshape(n_chunks * 2 * P, chunk_f)
        maps.append(m)
    return maps


_NC_CACHE = None


def kernel(x: np.ndarray, y: np.ndarray, z: np.ndarray) -> np.ndarray:
    global _NC_CACHE
    x = np.asarray(x, dtype=np.float32)
    y = np.asarray(y, dtype=np.float32)
    z = np.asarray(z, dtype=np.float32)
    if _NC_CACHE is None:
        _NC_CACHE = build_nc(1)
    res = bass_utils.run_bass_kernel_spmd(
        _NC_CACHE, make_in_maps(x, y, z), core_ids=list(range(N_CORES))
    )
    n_chunks = SHARD // F
    total = np.float64(0.0)
    for r in res.results:
        row = r["out"][0].astype(np.float64)
        total += row[:n_chunks].sum() - row[n_chunks:].sum()
    # sum_i relu(h_i + m) == sum_i max(h_i, -m) + m*N  (exact identity)
    total += np.float64(MARGIN) * N_TOTAL
    return np.float32(total)
